# revision 35
# baseline (speedup 1.0000x reference)
"""Causal self-attention Bass/Tile kernel for Trainium2, 8-core data-parallel.

Problem: B=8, T=1024, C=1024, H=16, D=64, fp32.
  qkv = x @ w_attn + b_attn; causal SDPA over 16 heads; out = y @ w_proj + b_proj

Sharding: batch (B=8) across the 8 NeuronCores - one batch element per core,
no collectives. Each core computes its full [T, C] output slice.

v2: bf16 operands (weights/activations converted on-chip, f32 psum accum),
exact-causal score widths, probs@V in natural orientation (full 128-row
contraction; ones-augmented V folds the softmax row-sum into the same
matmul), per-partition normalize on DVE, stripe DMA loads, deep
cross-phase interleaving to keep the PE busy.
"""

import sys
from contextlib import ExitStack

import numpy as np

import concourse.bass as bass
import concourse.tile as tile
from concourse import mybir
from concourse.bass_utils import run_bass_kernel_spmd
from concourse.masks import make_identity

F32 = mybir.dt.float32
F32R = mybir.dt.float32r
BF16 = mybir.dt.bfloat16
AF = mybir.ActivationFunctionType

# ---------------------------------------------------------------------------
# Workaround: this walrus build rejects instructions carrying more than one
# sem wait ("Too many sync wait commands").  Post-pass: move excess waits
# onto fresh single-wait NoOps inserted just before the instruction in its
# engine stream.
# ---------------------------------------------------------------------------
_MAX_WAITS = 1


def _split_sync_waits(nc, max_waits=_MAX_WAITS):
    uid = 0
    for f in nc.m.functions:
        for blk in f.blocks:
            insts = blk.instructions
            i = 0
            while i < len(insts):
                inst = insts[i]
                si = inst.sync_info
                if si is not None and len(si.on_wait) > max_waits:
                    waits = list(si.on_wait)
                    keep = waits[-max_waits:]
                    extra = waits[:-max_waits]
                    inst.sync_info = mybir.SyncInfo(
                        on_wait=keep, on_update=list(si.on_update)
                    )
                    pos = i
                    for j in range(0, len(extra), max_waits):
                        nop = mybir.InstNoOp(
                            name=f"wsplit-{uid}",
                            engine=inst.engine,
                            ins=[],
                            outs=[],
                            sync_info=mybir.SyncInfo(
                                on_wait=extra[j : j + max_waits], on_update=[]
                            ),
                        )
                        uid += 1
                        insts.insert(pos, nop)
                        pos += 1
                        i += 1
                i += 1


# ---------------------------------------------------------------------------
# Kernel build
# ---------------------------------------------------------------------------
N_CORES = 8
T = 1024
C = 1024
H = 16
D = C // H  # 64
C3 = 3 * C
P = 128  # partitions
NT = T // P      # 8 t-chunks
NCH = C // P     # 8 c-chunks
HPAIRS = H // 2  # 8 head pairs; pair hp = heads 2hp (parts 0-63), 2hp+1 (64-127)
SCALE = 1.0 / np.sqrt(D)


def _emit_kernel(nc, tc, ctx, x_d, wa_d, ba_d, wp_d, bp_d, out_d):
    # DRAM views with the c-contraction dim as partitions
    wa_v = wa_d.rearrange("(kc p) n -> p kc n", p=P)   # [128, 8, 3072]
    wp_v = wp_d.rearrange("(kc p) n -> p kc n", p=P)   # [128, 8, 1024]

    const = ctx.enter_context(tc.tile_pool(name="const", bufs=1))
    persist = ctx.enter_context(tc.tile_pool(name="persist", bufs=1))

    # --- constants -------------------------------------------------------
    ident_bf = const.tile([P, P], BF16)
    make_identity(nc, ident_bf)
    ident_f32 = const.tile([P, P], F32)
    make_identity(nc, ident_f32)
    ident_fr = const.tile([P, P], F32R)
    nc.gpsimd.tensor_copy(ident_fr, ident_f32)

    # trimask[p, f] = 1.0 where f >= p else 0.0   (S^T diag block: keep tq>=tk)
    tri_raw = const.tile([P, P], F32)
    nc.gpsimd.memset(tri_raw, 1.0)
    nc.gpsimd.affine_select(
        out=tri_raw, in_=tri_raw, compare_op=mybir.AluOpType.is_ge, fill=0.0,
        base=0, pattern=[[1, P]], channel_multiplier=-1,
    )
    trimask = const.tile([P, P], BF16)
    nc.gpsimd.tensor_copy(trimask, tri_raw)
    trimask2 = const.tile([P, 2, P], BF16)
    nc.gpsimd.tensor_copy(trimask2[:, 0, :], tri_raw)
    nc.gpsimd.tensor_copy(trimask2[:, 1, :], tri_raw)

    ones_bf = const.tile([33, P], BF16)
    nc.vector.memset(ones_bf[0:1, :], 1.0)
    nc.vector.memset(ones_bf[32:33, :], 1.0)

    # b_attn q/k part as [128, 16] f32 (partition p of column m = bias[m*128+p])
    ba_sb = const.tile([P, 2 * C // P], F32)
    b_raw = const.tile([33, C], F32)
    b_rows = const.tile([33, C], BF16)
    bv_row = b_rows[0:1, :]
    bp_row = b_rows[32:33, :]

    def load_biases():
        nc.sync.dma_start(out=ba_sb, in_=ba_d[0 : 2 * C].rearrange("(m p) -> p m", p=P))
        nc.sync.dma_start(out=b_raw[0:1, :], in_=ba_d[2 * C : 3 * C].rearrange("(o c) -> o c", o=1))
        nc.sync.dma_start(out=b_raw[32:33, :], in_=bp_d.rearrange("(o c) -> o c", o=1))
        nc.vector.tensor_copy(b_rows[0:1, :], b_raw[0:1, :])
        nc.vector.tensor_copy(b_rows[32:33, :], b_raw[32:33, :])

    load_biases()
    bias_bc = const.tile([P, 2, C], BF16, name="bias_bc")  # [*, 0]=b_v, [*, 1]=b_proj

    # --- persistent SBUF tensors ----------------------------------------
    xT = persist.tile([P, NCH, T], BF16, name="xT")          # [c-in-chunk, cch, t]
    vaug = persist.tile([P, NT, H, D + 1], BF16, name="vaug")  # [tk, tch, h, d|1]
    nc.vector.memset(vaug[:, :, :, D : D + 1], 1.0)
    yT = persist.tile([P, HPAIRS, T], BF16, name="yT")       # [c-in-pair, hp, tq]
    wp_bf = persist.tile([P, NCH, C], BF16, name="wp_bf")    # [c, kc, co]

    # staging pools
    stage = ctx.enter_context(tc.tile_pool(name="stage", bufs=2))
    wa_pool = ctx.enter_context(tc.tile_pool(name="wa_pool", bufs=2))
    qkT_pool = ctx.enter_context(tc.tile_pool(name="qkT_pool", bufs=2))
    e_pool = ctx.enter_context(tc.tile_pool(name="e_pool", bufs=2))
    qk_ps = ctx.enter_context(tc.tile_pool(name="qk_ps", bufs=2, space="PSUM"))

    # --- weight stripe loads --------------------------------------------
    def load_wv(n):
        for half in range(2):
            hs = slice(half * 4, (half + 1) * 4)
            nc.gpsimd.dma_start(
                out=wv_bf[:, n, hs, :],
                in_=wa_v[:, hs, 2 * C + n * 512 : 2 * C + (n + 1) * 512],
            )

    wa_tiles = {}

    def load_wa(m):
        """Stripe-load qk projection weights for m-chunk m (cols m*128..+128),
        casting f32 -> bf16 in the software-DGE DMA itself."""
        wa_bf = wa_pool.tile([P, NCH, P], BF16, tag="wa_bf", bufs=4, name=f"wa_bf_{m}")
        nc.gpsimd.dma_start(out=wa_bf, in_=wa_v[:, :, m * P : (m + 1) * P])
        wa_tiles[m] = wa_bf

    def load_wp(k):
        nc.gpsimd.dma_start(out=wp_bf[:, k : k + 1, :], in_=wp_v[:, k : k + 1, :])

    # --- qkT chunk: qkT[m][c', t] = (wa[:,m]^T x^T) + b ------------------
    qk_tiles = {}

    def emit_qk_part(m, g, psum_tile, on_act=False):
        """One [128,512] column group of qkT chunk m into psum_tile[:, 0:512]."""
        if m not in qk_tiles:
            tag = "qT" if m < NCH else "kT"
            qk_tiles[m] = qkT_pool.tile([P, T], BF16, tag=tag, bufs=3, name=f"qkT_{m}")
        qk = qk_tiles[m]
        wa_bf = wa_tiles[m]
        ps = psum_tile[:, 0:512]
        for k in range(NCH):
            nc.tensor.matmul(
                ps, lhsT=wa_bf[:, k, :], rhs=xT[:, k, g * 512 : (g + 1) * 512],
                start=(k == 0), stop=(k == NCH - 1),
            )
        dst = qk[:, g * 512 : (g + 1) * 512]
        if on_act:
            nc.scalar.activation(dst, ps, AF.Identity, bias=ba_sb[:, m : m + 1])
        else:
            nc.vector.tensor_scalar_add(dst, ps, ba_sb[:, m : m + 1])
        if g == 1:
            wa_tiles.pop(m)

    def emit_qk(m, pool, tag):
        for g in range(2):
            ps = pool.tile([P, 512], F32, tag=tag, name=f"qkps_{m}_{g}")
            emit_qk_part(m, g, ps, on_act=True)

    # === phase A: x load/convert/transpose + first qk + v ================
    ab = ExitStack()
    xstage = ab.enter_context(tc.tile_pool(name="xstage", bufs=2))
    wvbf_pool = ab.enter_context(tc.tile_pool(name="wvbf", bufs=1))
    wv_bf = wvbf_pool.tile([P, 2, NCH, 512], BF16, name="wv_bf")  # [c, n, kc, 512]
    tp_ps = ab.enter_context(tc.tile_pool(name="tp_ps", bufs=2, space="PSUM"))
    qk_ps = ab.enter_context(tc.tile_pool(name="qk_ps", bufs=2, space="PSUM"))
    v_ps = ab.enter_context(tc.tile_pool(name="v_ps", bufs=4, space="PSUM"))

    def xT_chunk(tch):
        xr = xstage.tile([P, C], F32R, tag="x_raw", bufs=6, name=f"x_raw_{tch}")
        nc.sync.dma_start(out=xr[:, 0:512], in_=x_d[tch * P : (tch + 1) * P, 0:512])
        nc.sync.dma_start(out=xr[:, 512:C], in_=x_d[tch * P : (tch + 1) * P, 512:C])
        for half in range(2):
            ps = tp_ps.tile([P, 512], F32R, tag="tp", name=f"tp_{tch}_{half}")
            for j in range(4):
                cch = half * 4 + j
                nc.tensor.transpose(
                    ps[:, j * P : (j + 1) * P],
                    xr[:, cch * P : (cch + 1) * P], ident_fr,
                )
            nc.vector.tensor_copy(
                xT[:, half * 4 : (half + 1) * 4, tch * P : (tch + 1) * P],
                ps.rearrange("p (a b) -> p a b", a=4),
            )

    def v_chunk(tch, n):
        ps = v_ps.tile([P, 512], F32, tag="vps", name=f"vps_{tch}_{n}")
        for k in range(NCH):
            nc.tensor.matmul(
                ps, lhsT=xT[:, k, tch * P : (tch + 1) * P],
                rhs=wv_bf[:, n, k, :],
                start=(k == 0), stop=(k == NCH - 1),
            )
        nc.vector.tensor_add(
            vaug[:, tch, n * 8 : (n + 1) * 8, 0:D],
            ps.rearrange("p (h d) -> p h d", h=8),
            bias_bc[:, 0, n * 512 : (n + 1) * 512].rearrange("p (h d) -> p h d", h=8),
        )

    # interleave x chunks with weight stripe loads (separate DMA queues)
    xT_chunk(0)
    load_wv(0)
    xT_chunk(1)
    load_wa(0)
    load_wa(NCH)
    xT_chunk(2)
    xT_chunk(3)
    for bi, row, plo in ((0, bv_row, 0), (1, bp_row, 32)):
        for n in range(2):
            bps = qk_ps.tile([P, 512], F32, tag="qkps", name=f"bps_{bi}_{n}")
            nc.tensor.matmul(
                bps, lhsT=ones_bf[plo : plo + 1, :],
                rhs=row[0:1, n * 512 : (n + 1) * 512],
                start=True, stop=True,
                tile_position=(plo, 0) if plo else None,
            )
            nc.vector.tensor_copy(bias_bc[:, bi, n * 512 : (n + 1) * 512], bps)
    emit_qk_part(0, 0, qk_ps.tile([P, 512], F32, tag="qkps", name="qkps_0_0"), on_act=True)
    emit_qk_part(NCH, 0, qk_ps.tile([P, 512], F32, tag="qkps", name="qkps_8_0"), on_act=True)
    for tch in range(4, NT):
        xT_chunk(tch)
    emit_qk_part(0, 1, qk_ps.tile([P, 512], F32, tag="qkps", name="qkps_0_1"), on_act=True)
    emit_qk_part(NCH, 1, qk_ps.tile([P, 512], F32, tag="qkps", name="qkps_8_1"), on_act=True)
    load_wa(1)
    load_wa(NCH + 1)
    load_wv(1)
    # v sweep n=0 (needs wv stripe 0), then qk pair 1, then v sweep n=1
    for tch in range(NT):
        v_chunk(tch, 0)
    emit_qk(1, qk_ps, "qkps")
    load_wa(2)
    load_wa(NCH + 2)
    for tch in range(NT):
        v_chunk(tch, 1)
    emit_qk(NCH + 1, qk_ps, "qkps")
    ab.close()

    # === phase C: attention, pair-by-pair ================================
    attn = ExitStack()
    s_ps = attn.enter_context(tc.tile_pool(name="s_ps", bufs=3, space="PSUM"))
    pv_pool = attn.enter_context(tc.tile_pool(name="pv_ps", bufs=1, space="PSUM"))
    yt_pool = attn.enter_context(tc.tile_pool(name="yt_ps", bufs=1, space="PSUM"))
    ynat_pool = attn.enter_context(tc.tile_pool(name="ynat", bufs=2))
    pv_bank = pv_pool.tile([P, 6, D + 1], F32, name="pv_bank")
    yt_bank = yt_pool.tile([P, 4, P], BF16, name="yt_bank")
    pacc = persist.tile([P, NT, C], F32, name="pacc")  # proj partial sums
    pacc_k = [0] * NT  # k-chunks accumulated into pacc[:, m, :] so far

    def proj_partial(m, k_hi):
        """Accumulate proj m-chunk over k in [pacc_k[m], k_hi) into pacc."""
        k_lo = pacc_k[m]
        if k_hi <= k_lo:
            return
        for n in range(2):
            sp = s_ps.tile([P, T], F32, tag="sps", name=f"ppart_{m}_{n}_{k_hi}")
            ps = sp[:, 0:512]
            for k in range(k_lo, k_hi):
                nc.tensor.matmul(
                    ps, lhsT=yT[:, k, m * P : (m + 1) * P],
                    rhs=wp_bf[:, k, n * 512 : (n + 1) * 512],
                    start=(k == k_lo), stop=(k == k_hi - 1),
                )
            if k_lo == 0:
                nc.vector.tensor_add(
                    pacc[:, m, n * 512 : (n + 1) * 512], ps,
                    bias_bc[:, 1, n * 512 : (n + 1) * 512],
                )
            else:
                nc.vector.tensor_add(
                    pacc[:, m, n * 512 : (n + 1) * 512],
                    pacc[:, m, n * 512 : (n + 1) * 512], ps,
                )
        pacc_k[m] = k_hi

    def proj_final(m):
        """Finish proj m-chunk (remaining k + bias), add into pacc, DMA out."""
        k_lo = pacc_k[m]
        for n in range(2):
            sp = s_ps.tile([P, T], F32, tag="sps", name=f"pfin_{m}_{n}")
            ps = sp[:, 0:512]
            for k in range(k_lo, NCH):
                nc.tensor.matmul(
                    ps, lhsT=yT[:, k, m * P : (m + 1) * P],
                    rhs=wp_bf[:, k, n * 512 : (n + 1) * 512],
                    start=(k == k_lo), stop=(k == NCH - 1),
                )
            nc.vector.tensor_add(
                pacc[:, m, n * 512 : (n + 1) * 512],
                pacc[:, m, n * 512 : (n + 1) * 512], ps,
            )
        pacc_k[m] = NCH + 1
        nc.sync.dma_start(out=out_d[m * P : (m + 1) * P, :], in_=pacc[:, m, :])

    # filler queue: closures giving the PE off-critical-path work, consumed
    # at slots i in {1,2,3,5} of each attention pair (4 per pair, 32 total):
    # qk column-groups for pairs 2..7 (24), then proj partials (8).
    filler = []
    for p in range(2, HPAIRS):
        for m in (p, NCH + p):
            for g in range(2):
                filler.append(
                    lambda m=m, g=g: emit_qk_part(
                        m, g,
                        s_ps.tile([P, T], F32, tag="sps", name=f"qkf_{m}_{g}"),
                    )
                )
    for m in range(4):
        filler.append(lambda m=m: proj_partial(m, 6))
    for m in range(4, NT):
        filler.append(lambda m=m: proj_partial(m, 7))

    def make_pair(hp):
        hA, hB = 2 * hp, 2 * hp + 1
        q_tile, k_tile = qk_tiles.pop(hp), qk_tiles.pop(NCH + hp)
        e_tiles = {}  # (i, head_idx) -> [128, W] AP

        def scores(i):
            W = T - i * P
            if i < 4:
                for hi, plo in ((0, 0), (1, 64)):
                    sp = s_ps.tile([P, T], F32, tag="sps", name=f"sps_{hp}_{i}_{hi}")
                    for w0 in range(0, W, 512):
                        wl = min(512, W - w0)
                        nc.tensor.matmul(
                            sp[:, w0 : w0 + wl],
                            lhsT=k_tile[plo : plo + 64, i * P : (i + 1) * P],
                            rhs=q_tile[plo : plo + 64, i * P + w0 : i * P + w0 + wl],
                            tile_position=(plo, 0),
                        )
                    e = e_pool.tile([P, W], BF16, tag=f"e{i}", bufs=4, name=f"e_{hp}_{i}_{hi}")
                    e_tiles[(i, hi)] = e
                    nc.scalar.activation(e[:, 0:W], sp[:, 0:W], AF.Exp, scale=float(SCALE))
                    nc.vector.tensor_mul(e[:, 0:P], e[:, 0:P], trimask)
            else:
                # both heads share one psum tile / one exp / one mask
                sp = s_ps.tile([P, T], F32, tag="sps", name=f"sps_{hp}_{i}")
                for hi, plo in ((0, 0), (1, 64)):
                    nc.tensor.matmul(
                        sp[:, hi * 512 : hi * 512 + W],
                        lhsT=k_tile[plo : plo + 64, i * P : (i + 1) * P],
                        rhs=q_tile[plo : plo + 64, i * P : i * P + W],
                        tile_position=(plo, 0),
                    )
                ep = e_pool.tile([P, 2, W], BF16, tag=f"ep{i}", bufs=2, name=f"ep_{hp}_{i}")
                e_tiles[(i, 0)] = ep[:, 0, :]
                e_tiles[(i, 1)] = ep[:, 1, :]
                nc.scalar.activation(
                    ep, sp.rearrange("p (a w) -> p a w", a=2)[:, :, 0:W],
                    AF.Exp, scale=float(SCALE),
                )
                nc.vector.tensor_mul(ep[:, :, 0:P], ep[:, :, 0:P], trimask2)

        def pv_norm_trans(ti):
            # accumulate pv for tq-chunk ti over tk-chunks 0..ti, both heads
            for hi, h in enumerate((hA, hB)):
                slot = 3 * hi + ti % 3
                acc = pv_bank[:, slot, :]
                for ii in range(ti + 1):
                    nc.tensor.matmul(
                        acc, lhsT=e_tiles[(ii, hi)][:, (ti - ii) * P : (ti - ii + 1) * P],
                        rhs=vaug[:, ii, h, :],
                        start=(ii == 0), stop=(ii == ti),
                    )
            yn = ynat_pool.tile([P, P], BF16, tag="ynat", name=f"ynat_{hp}_{ti}")
            rcp = ynat_pool.tile([P, 2], F32, tag="rcp", name=f"rcp_{hp}_{ti}")
            sA = ti % 3
            nc.vector.reciprocal(
                rcp, pv_bank[:, sA::3, D : D + 1].rearrange("p a b -> p (a b)")
            )
            for hi in range(2):
                slot = 3 * hi + sA
                nc.vector.tensor_scalar_mul(
                    yn[:, hi * D : (hi + 1) * D],
                    pv_bank[:, slot, 0:D],
                    rcp[:, hi : hi + 1],
                )
            nc.tensor.transpose(yt_bank[:, ti % 4, :], yn, ident_bf)
            if ti % 4 == 3:
                nc.vector.tensor_copy(
                    yT[:, hp, (ti - 3) * P : (ti + 1) * P],
                    yt_bank.rearrange("p a b -> p (a b)"),
                )

        return scores, pv_norm_trans

    prev_pv = None
    for hp in range(HPAIRS):
        scores, pv_nt = make_pair(hp)
        for i in range(NT):
            # previous pair's last two pv groups trail into this pair's start,
            # covering the Act exp backlog at the boundary
            if i == 0 and prev_pv is not None:
                prev_pv(NT - 2)
            if i == 1 and prev_pv is not None:
                prev_pv(NT - 1)
            if i in (1, 2, 3, 5) and filler:
                filler.pop(0)()
            scores(i)
            if i >= 2:
                pv_nt(i - 2)
            # wa stripes for pair hp+3, one pair ahead of their consumption
            if i == 4 and hp + 3 < HPAIRS:
                load_wa(hp + 3)
            elif i == 6 and hp + 3 < HPAIRS:
                load_wa(NCH + hp + 3)
            if i == 7:
                load_wp(hp)
        prev_pv = pv_nt
    prev_pv(NT - 2)
    prev_pv(NT - 1)
    attn.close()

    # === phase D: finish proj (remaining k-chunks + bias), DMA from pacc =
    with tc.tile_pool(name="proj_ps", bufs=4, space="PSUM") as proj_ps:
        for m in range(NT):
            for n in range(2):
                ps = proj_ps.tile([P, 512], F32, tag="pps", name=f"pps_{m}_{n}")
                k_lo = pacc_k[m]
                for k in range(k_lo, NCH):
                    nc.tensor.matmul(
                        ps, lhsT=yT[:, k, m * P : (m + 1) * P],
                        rhs=wp_bf[:, k, n * 512 : (n + 1) * 512],
                        start=(k == k_lo), stop=(k == NCH - 1),
                    )
                nc.vector.tensor_add(
                    pacc[:, m, n * 512 : (n + 1) * 512],
                    pacc[:, m, n * 512 : (n + 1) * 512], ps,
                )
            nc.sync.dma_start(out=out_d[m * P : (m + 1) * P, :], in_=pacc[:, m, :])


def build_nc(n_cores=N_CORES, reps=1):
    nc = bass.Bass("TRN2", target_bir_lowering=False, debug=False, num_devices=n_cores)
    x_d = nc.dram_tensor("x", [T, C], F32R, kind="ExternalInput").ap()
    wa_d = nc.dram_tensor("w_attn", [C, C3], F32, kind="ExternalInput").ap()
    ba_d = nc.dram_tensor("b_attn", [C3], F32, kind="ExternalInput").ap()
    wp_d = nc.dram_tensor("w_proj", [C, C], F32, kind="ExternalInput").ap()
    bp_d = nc.dram_tensor("b_proj", [C], F32, kind="ExternalInput").ap()
    out_d = nc.dram_tensor("out", [T, C], F32, kind="ExternalOutput").ap()
    with tile.TileContext(nc) as tc:
        with nc.allow_low_precision(reason="bf16 matmul operands are intentional"):
            for _ in range(reps):
                with ExitStack() as ctx:
                    _emit_kernel(nc, tc, ctx, x_d, wa_d, ba_d, wp_d, bp_d, out_d)
    _split_sync_waits(nc)
    return nc


_NC_CACHE = {}


def _get_nc(n_cores=N_CORES):
    if n_cores not in _NC_CACHE:
        _NC_CACHE[n_cores] = build_nc(n_cores)
    return _NC_CACHE[n_cores]


def kernel(x, attn_mask, w_attn, b_attn, w_proj, b_proj):
    """Full inputs in, full output out. attn_mask is causal (hardcoded)."""
    x = np.ascontiguousarray(np.asarray(x, dtype=np.float32))
    w_attn = np.ascontiguousarray(np.asarray(w_attn, dtype=np.float32))
    b_attn = np.ascontiguousarray(np.asarray(b_attn, dtype=np.float32))
    w_proj = np.ascontiguousarray(np.asarray(w_proj, dtype=np.float32))
    b_proj = np.ascontiguousarray(np.asarray(b_proj, dtype=np.float32))
    B = x.shape[0]
    assert B == N_CORES and x.shape == (B, T, C)

    nc = _get_nc(N_CORES)
    in_maps = [
        {"x": x[b], "w_attn": w_attn, "b_attn": b_attn,
         "w_proj": w_proj, "b_proj": b_proj}
        for b in range(B)
    ]
    res = run_bass_kernel_spmd(nc, in_maps, core_ids=list(range(N_CORES)))
    return np.stack([res.results[b]["out"] for b in range(B)], axis=0)
